# revision 36
# baseline (speedup 1.0000x reference)
"""Trainium2 Bass kernel for nn_CapsShapeLayer (capsule dynamic routing).

Reference computation:
    u_hat[b,r,c,o] = sum_i W[r,c,o,i] * x[b,r,i]
    3 routing iterations:
        c = softmax(b_logits, axis=r)
        s[b,c,o] = sum_r c[r,c] * u_hat[b,r,c,o]
        v = squash(s)                                    (elementwise)
        b_logits += mean_b <u_hat[b,r,c,:], v[b,c,:]>

Kernel strategy (u_hat never materialized; ONE device collective total):
  * Iteration 0's softmax is uniform (b == 0), so s_0 is the routing-
    independent full contraction sum_{(r,i)} Wt[(r,i),(o,c)] * Xt[(r,i),b].
    Every core computes it REDUNDANTLY (72 pipelined K=128 fp8 matmuls,
    overlapped with the input DMA) -- no collective needed for iter 0.
    fp8 operands halve the DMA-bound front; the quantization noise only
    perturbs the routing logits and washes out (~3.8e-3 final rel err).
  * Each core's wt/xt arrive K-tile-ROTATED so tiles 0:9 are its own
    route shard; iterations 1-2 then run the r-sharded path (agreement,
    b-logits, wc scaling all local) identically on every core.
  * ONE bf16 AllGather (iter 1) carries the s~ partial plus local
    exp-sums ([128, 272] = 70KB); each core tree-sums the 8 slices in
    f32 on DVE.  Softmax divide is deferred past the cross-core sum:
    s = s~/S, S[c] = global sum_r exp(b[r,c]).  (The collective cannot
    start before ~62us regardless of trigger time: the cc runtime's
    entry barrier runs ~21.7->50us on every execution and the first op
    pays ~11us of ncfw setup -- so the front has ~8us of slack.)
  * The final iteration ships raw f32 partials to the host, which does
    the last sum + softmax divide + squash in the unshard step.
  * Matmul operands are fp8 (iter 0) / bf16 (iters 1-2); free dims are
    (o, c)-ordered so per-c broadcasts land on the DVE fast path.
  * Routing logits live as b[16, (t,c)] (q = r%16 on partitions, local
    tile t and capsule c on cols): softmax bookkeeping is pure matmul
    with stationary operands loaded once (ones/rep16/red16 constants).
  * Agreement: G_t = Xb_t^T @ v (9 K=128 bf16 matmuls), P = Wt .* G
    (DVE), o-reduce (DVE), then ONE i-reduce matmul red16^T @ q (1/B
    folded into red16) writing a[16,(t,c)] straight in the b layout.
  * squash: v = s~|s~| / (S^2 + s~^2); Square/Sign on ScalarE, the
    reciprocal is the fast DVE approximation.
"""

import sys

for _p in ("/opt/trn_rl_repo",):
    if _p not in sys.path:
        sys.path.insert(0, _p)

import numpy as np
import ml_dtypes

import concourse.bass as bass
import concourse.bacc as bacc
import concourse.mybir as mybir
import concourse.tile as tile
from concourse.bass_utils import run_bass_kernel_spmd

F32 = mybir.dt.float32
BF16 = mybir.dt.bfloat16
F8 = mybir.dt.float8e4
AX = mybir.AxisListType
ALU = mybir.AluOpType
ACT = mybir.ActivationFunctionType
BF = ml_dtypes.bfloat16
F8H = ml_dtypes.float8_e4m3

B = 128          # batch
R = 1152         # routes (input capsules)
C = 16           # output capsules
O = 16           # output capsule dim
I = 8            # input capsule dim
CO = C * O       # 256
NCORES = 8
RS = R // NCORES          # 144 local routes
KL = RS * I               # 1152 local contraction
KT = KL // 128            # 9 local K-tiles of 128
KTG = R * I // 128        # 72 global K-tiles
NITER = 3
S0 = float(R)             # global softmax denom at iteration 0 (b == 0)
EW = CO + C               # payload width with exp-sum columns

WCH = 8                   # wt DMA chunks (9 tiles each; a small final
                          # chunk shortens the matmul tail after DMA-end)
XCH = 2                   # xt DMA chunks (36 tiles each)
WTC = KTG // WCH          # 18
XTC = KTG // XCH          # 36

_CACHED = None


def _make_consts():
    """Constant matrix packed into one [128, 272] bf16 input.

    cols 0:16     red16[p, m] = 1/B if m == p//8   (i-reduction + batch mean)
    cols 16:144   rep16[q, p'] = 1 if q == p'//8   (rows 0:16; c-replication)
    cols 144:272  ones         (rows 0:16; exp-sum partition broadcast)
    """
    cst = np.zeros((128, 272), np.float32)
    p = np.arange(128)
    cst[p, p // 8] = 1.0 / B
    cst[p // 8, 16 + p] = 1.0
    cst[0:16, 144:272] = 1.0
    return cst.astype(BF)


def _prep_inputs(x, W):
    """Host-side relayout. Returns list of 8 per-core input dicts.

    Every core gets the FULL Wt/Xt in fp8 (for the replicated iter-0
    contraction -- quantization noise there only perturbs the routing
    logits and washes out, ~3.8e-3 final rel err), K-tile-rotated so
    tiles 0:9 are its own route shard, plus its LOCAL shard in bf16 for
    iterations 1-2 and the agreement."""
    x = np.asarray(x, dtype=np.float32)
    W = np.asarray(W, dtype=np.float32)
    Wr = W.reshape(R, C, O, I)
    cst = _make_consts()
    # global K-tiled layouts, contraction index (r, i) row-major
    wt_g = Wr.transpose(0, 3, 2, 1).reshape(KTG, 128, CO)
    xt_g = x.transpose(1, 2, 0).reshape(KTG, 128, B)
    wt8_g = wt_g.astype(F8H)
    xt8_g = xt_g.astype(F8H)
    in_maps = []
    for k in range(NCORES):
        rot = np.roll(np.arange(KTG), -KT * k)
        loc = slice(KT * k, KT * (k + 1))
        wt8 = np.ascontiguousarray(wt8_g[rot].transpose(1, 0, 2))
        xt8 = np.ascontiguousarray(xt8_g[rot].transpose(1, 0, 2))
        wt = np.ascontiguousarray(wt_g[loc].transpose(1, 0, 2).astype(BF))
        xb = np.ascontiguousarray(
            x[:, k * RS : (k + 1) * RS, :].reshape(B, KL).astype(BF)
        )
        in_maps.append({
            "wt8_in": wt8, "xt8_in": xt8,
            "wt_in": wt, "xb_in": xb, "cst_in": cst,
        })
    return in_maps


def _build_nc():
    nc = bacc.Bacc(
        "TRN2",
        target_bir_lowering=False,
        debug=False,
        num_devices=NCORES,
    )
    wt8_d = nc.dram_tensor("wt8_in", [128, KTG, CO], F8, kind="ExternalInput")
    xt8_d = nc.dram_tensor("xt8_in", [128, KTG, B], F8, kind="ExternalInput")
    wt_d = nc.dram_tensor("wt_in", [128, KT, CO], BF16, kind="ExternalInput")
    xb_d = nc.dram_tensor("xb_in", [B, KL], BF16, kind="ExternalInput")
    cst_d = nc.dram_tensor("cst_in", [128, 272], BF16, kind="ExternalInput")
    v_d = nc.dram_tensor("v_out", [B, EW], F32, kind="ExternalOutput")

    rg = [list(range(NCORES))]

    with tile.TileContext(nc) as tc:
        with (
            tc.tile_pool(name="persist", bufs=1) as pp,
            tc.tile_pool(name="work", bufs=2) as wp,
            tc.tile_pool(name="ps_s", bufs=1, space="PSUM") as pool_ps_s,
            tc.tile_pool(name="ps_g", bufs=2, space="PSUM") as pool_ps_g,
            tc.tile_pool(name="ps_small", bufs=3, space="PSUM") as pool_ps_small,
            tc.tile_pool(name="dram", bufs=1, space="DRAM") as dp,
        ):
            # ---- persistent SBUF state ----
            # chunked fp8 tiles so each iter-0 matmul waits only on its
            # own chunk's DMA, not the whole wt/xt load
            wtc = [
                pp.tile([128, WTC, CO], F8, name=f"wtc{j}")
                for j in range(WCH)
            ]
            xtc = [
                pp.tile([128, XTC, B], F8, name=f"xtc{j}")
                for j in range(XCH)
            ]
            wt_sb = pp.tile([128, KT, CO], BF16, name="wt_sb")
            xt_sb = pp.tile([128, KT, B], BF16, name="xt_sb")
            wc_sb = pp.tile([128, KT, CO], BF16, name="wc_sb")
            xb_sb = pp.tile([B, KL], BF16, name="xb_sb")
            cst_sb = pp.tile([128, 272], BF16, name="cst_sb")
            b_sb = pp.tile([16, RS], F32, name="b_sb")

            # cst first (small); xt8 before wt8 (every matmul's
            # stationary); bf16 locals + xb last (first used ~28us in)
            nc.sync.dma_start(cst_sb[:], cst_d[:])
            for j in range(XCH):
                nc.sync.dma_start(
                    xtc[j][:], xt8_d[:, j * XTC : (j + 1) * XTC]
                )
            for j in range(WCH):
                nc.sync.dma_start(
                    wtc[j][:], wt8_d[:, j * WTC : (j + 1) * WTC]
                )
            nc.sync.dma_start(wt_sb[:], wt_d[:])
            nc.sync.dma_start(xb_sb[:], xb_d[:])
            # xt_loc is the per-tile TRANSPOSE of xb -- derive it on-chip
            # via the XBAR instead of DMAing a second 0.3MB layout from
            # HBM (first use is the s~_1 matmul at ~41us; plenty of slack)
            for t in range(KT):
                nc.sync.dma_start_transpose(
                    xt_sb[:, t, :], xb_sb[:, 128 * t : 128 * (t + 1)]
                )
            nc.vector.memset(b_sb[:], 0.0)

            red16 = cst_sb[:, 0:16]          # [128, 16]  (1/B weighted)
            rep16 = cst_sb[0:16, 16:144]     # [16, 128]
            ones16 = cst_sb[0:16, 144:272]   # [16, 128]
            wt_loc = wt_sb                   # this core's route shard

            for it in range(NITER):
                first, last = it == 0, it == NITER - 1

                # s~ partial: psum[b, oc] = sum_t Xt_t^T @ Wmm_t
                ps_s = pool_ps_s.tile([B, CO], F32, name="ps_s", tag="s")
                if first:
                    # replicated full contraction (uniform softmax)
                    for t in range(KTG):
                        nc.tensor.matmul(
                            ps_s[:],
                            xtc[t // XTC][:, t % XTC, :],
                            wtc[t // WTC][:, t % WTC, :],
                            start=(t == 0),
                            stop=(t == KTG - 1),
                        )
                else:
                    for t in range(KT):
                        nc.tensor.matmul(
                            ps_s[:],
                            xt_sb[:, t, :],
                            wc_sb[:, t, :],
                            start=(t == 0),
                            stop=(t == KT - 1),
                        )

                if last:
                    # raw f32 partial straight to the host: final sum +
                    # squash happen in the host-side unshard (DMA cannot
                    # read PSUM, so stage through SBUF in two halves)
                    stF = wp.tile([B, EW], F32, name="stF", tag="stF")
                    nc.vector.tensor_copy(stF[:, 0:128], ps_s[:, 0:128])
                    nc.sync.dma_start(v_d[:, 0:128], stF[:, 0:128])
                    nc.vector.tensor_copy(stF[:, 128:CO], ps_s[:, 128:CO])
                    nc.scalar.copy(stF[:, CO:EW], S8_sb[:])
                    nc.sync.dma_start(v_d[:, 128:EW], stF[:, 128:EW])
                    continue

                if first:
                    # no collective: every core holds the full s~_0
                    sm = wp.tile([B, CO], F32, name="sm0", tag="sm0")
                    nc.vector.tensor_copy(sm[:], ps_s[:])
                else:
                    # bf16 payload [128, 272]: s~ partial + local exp-sums.
                    # (AllReduce/RDH ~12us; AllGather was tried and picked
                    # RDH too at this size, costing ~18us -- keep AR.)
                    st = wp.tile([B, EW], BF16, name="st", tag=f"st{it}")
                    nc.vector.tensor_copy(st[:, 0:CO], ps_s[:])
                    nc.scalar.copy(st[:, CO:EW], S8_sb[:])
                    cc_in = dp.tile([B, EW], BF16, name=f"cc_in{it}")
                    cc_out = dp.tile([B, EW], BF16, name=f"cc_out{it}",
                                     addr_space="Shared")
                    nc.sync.dma_start(cc_in[:], st[:])
                    nc.gpsimd.collective_compute(
                        "AllReduce", ALU.add, replica_groups=rg,
                        ins=[cc_in[:].opt()], outs=[cc_out[:].opt()],
                    )
                    sm = wp.tile([B, EW], BF16, name="sm", tag="sm")
                    nc.sync.dma_start(sm[:], cc_out[:])

                # squash: v = s~|s~| / (S^2 + s~^2)
                #       = (s~^2 * sign(s~)) * recip(S^2 + s~^2)
                qt = wp.tile([B, CO], F32, name="qt", tag="qt")
                sg = wp.tile([B, CO], BF16, name="sg", tag="sg")
                q2 = wp.tile([B, CO], F32, name="q2", tag="q2")
                rec = wp.tile([B, CO], F32, name="rec", tag="rec")
                m = wp.tile([B, CO], F32, name="m", tag="m")
                vg = wp.tile([B, CO], BF16, name="vg", tag="vg")
                # qt on DVE (it gates the whole q2/rec chain); Sign runs on
                # ScalarE in parallel, m on GpSimd
                if first:
                    nc.vector.tensor_mul(qt[:], sm[:, 0:CO], sm[:, 0:CO])
                    nc.scalar.activation(sg[:], sm[:, 0:CO], ACT.Sign)
                    nc.vector.tensor_scalar_add(q2[:], qt[:], S0 * S0)
                else:
                    # sS2 on ScalarE (16-col square, emitted BEFORE sg so
                    # ScalarE runs it first) -- keeps the DVE chain at
                    # qt -> q2 -> rec -> vg
                    sS2 = wp.tile([B, C], F32, name="sS2", tag="sS2")
                    nc.scalar.square(sS2[:], sm[:, CO:EW])
                    nc.vector.tensor_mul(qt[:], sm[:, 0:CO], sm[:, 0:CO])
                    nc.scalar.activation(sg[:], sm[:, 0:CO], ACT.Sign)
                    nc.vector.tensor_add(
                        q2[:].rearrange("b (o c) -> b o c", c=C),
                        qt[:].rearrange("b (o c) -> b o c", c=C),
                        sS2[:, None, :].broadcast_to([B, O, C]),
                    )
                nc.vector.reciprocal_approx_fast(rec[:], q2[:])
                # m on GpSimd: runs concurrently with the DVE reciprocal
                nc.gpsimd.tensor_mul(m[:], qt[:], sg[:])
                nc.vector.tensor_mul(vg[:], m[:], rec[:])

                # agreement: G_t = Xb_t^T @ v;  P = Wt .* G;  o-reduce
                # (only down to PAIRS of o-planes -- the last o-level rides
                # the i-reduce matmul as extra N and a tiny [16,288] add).
                # Chunked (4,4,1): the final 1-tile chunk's short DVE chain
                # unblocks the downstream i-reduce matmul ~1us earlier.
                # (GpSimd tree offload was tried: its ~0.9us/op latency on
                # the dependency chain cost +2us per agreement.)
                q_sb = wp.tile([128, KT, 2, C], BF16, name="q_sb", tag="q_sb")
                for c0 in range(0, KT, 4):
                    nt = min(4, KT - c0)
                    ps_g = pool_ps_g.tile([128, 4, CO], F32, name="ps_g", tag="g")
                    for j in range(nt):
                        t = c0 + j
                        nc.tensor.matmul(
                            ps_g[:, j, :],
                            xb_sb[:, 128 * t : 128 * (t + 1)],
                            vg[:],
                            start=True, stop=True,
                        )
                    # ScalarE stages G from PSUM to bf16 (it idles during
                    # the G section) so the DVE multiply runs all-bf16 at
                    # 2 elem/cycle instead of reading PSUM f32 at 1/cycle
                    # (PSUM-direct was tried: +1.8us of DVE serial time)
                    gb = wp.tile([128, 4, CO], BF16, name="gb", tag="gb")
                    nc.scalar.copy(gb[:, 0:nt], ps_g[:, 0:nt])
                    p_bf = wp.tile([128, 4, O, C], BF16, name="p_bf", tag="p_bf")
                    nc.vector.tensor_mul(
                        p_bf[:, 0:nt].rearrange("p t o c -> p t (o c)"),
                        wt_loc[:, c0 : c0 + nt],
                        gb[:, 0:nt],
                    )
                    # o-sum as an in-place binary tree: every level reads
                    # and writes contiguous inner-c runs (the strided-inner
                    # tensor_reduce path is ~1.7x slower on DVE)
                    for hw in (8, 4):
                        nc.vector.tensor_add(
                            p_bf[:, 0:nt, 0:hw],
                            p_bf[:, 0:nt, 0:hw],
                            p_bf[:, 0:nt, hw : 2 * hw],
                        )
                    nc.vector.tensor_add(
                        q_sb[:, c0 : c0 + nt],
                        p_bf[:, 0:nt, 0:2, :],
                        p_bf[:, 0:nt, 2:4, :],
                    )

                # a[q,(t,c)] = red16^T @ q  (i-reduce + 1/B), b += a.
                # The leftover o-PAIR is folded by PSUM accumulation across
                # two matmuls, so b needs only ONE DVE add, not two.
                ps_a = pool_ps_small.tile([16, KT, C], F32, name="ps_a", tag="sp")
                for j in range(2):
                    nc.tensor.matmul(
                        ps_a[:],
                        red16,
                        q_sb[:, :, j, :],
                        start=(j == 0), stop=(j == 1),
                    )
                bv = b_sb[:].rearrange("q (t c) -> q t c", c=C)
                if first:
                    nc.vector.tensor_copy(bv, ps_a[:])
                else:
                    nc.vector.tensor_add(bv, bv, ps_a[:])
                eb = wp.tile([16, RS], BF16, name="eb", tag="eb")
                nc.scalar.activation(eb[:], b_sb[:], ACT.Exp)

                # c-replication first -- it alone gates the wc -> s~ chain;
                # the exp-sum matmul (payload-only) follows on the PE FIFO.
                # crb is staged to bf16 in two pieces (PSUM-direct reads in
                # the wc multiply cost ~0.4us/instruction extra on DVE).
                ps_c = pool_ps_small.tile([128, RS], F32, name="ps_c", tag="sp")
                nc.tensor.matmul(ps_c[:], rep16, eb[:], start=True, stop=True)
                crb = wp.tile([128, RS], BF16, name="crb", tag="crb")
                nc.scalar.copy(crb[:, 0:48], ps_c[:, 0:48])
                nc.scalar.copy(crb[:, 48:RS], ps_c[:, 48:RS])

                # local exp-sum, broadcast to all partitions via ones-matmul
                ps_S = pool_ps_small.tile([128, RS], F32, name="ps_S", tag="sp")
                nc.tensor.matmul(ps_S[:], ones16, eb[:], start=True, stop=True)
                S8_sb = wp.tile([128, C], F32, name="S8", tag="S8")
                nc.vector.tensor_reduce(
                    S8_sb[:],
                    ps_S[:].rearrange("p (t c) -> p c t", t=KT),
                    axis=AX.X, op=ALU.add,
                )

                # wc = wt .* crep (broadcast over the middle o dim), chunked
                # so the next iteration's s~ matmuls start while later
                # chunks scale
                for c0 in range(0, KT, 3):
                    nt = min(3, KT - c0)
                    nc.vector.tensor_mul(
                        wc_sb[:, c0 : c0 + nt].rearrange(
                            "p t (o c) -> p t o c", c=C
                        ),
                        wt_loc[:, c0 : c0 + nt].rearrange(
                            "p t (o c) -> p t o c", c=C
                        ),
                        crb[:, C * c0 : C * (c0 + nt)].rearrange(
                            "p (t c) -> p t c", c=C
                        )[:, :, None, :].broadcast_to([128, nt, O, C]),
                    )

    nc.compile()
    return nc


def _get_nc():
    global _CACHED
    if _CACHED is None:
        _CACHED = _build_nc()
    return _CACHED


def _postprocess(outs):
    """Host-side unshard of the final routing iteration: sum the per-core
    s~ partials and exp-sums, then apply the deferred softmax divide and
    squash (v = s~|s~| / (S^2 + s~^2)).  Device layout is (o, c)-ordered,
    transpose back at the end."""
    tot = np.stack([np.asarray(o) for o in outs]).sum(axis=0)
    st = tot[:, 0:CO]
    S = tot[0, CO:EW]
    S2 = np.tile(S * S, O)[None, :]
    q = st * st
    v = (q * np.sign(st) / (S2 + q)).astype(np.float32)
    return np.ascontiguousarray(v.reshape(B, O, C).transpose(0, 2, 1))


def kernel(x, W):
    nc = _get_nc()
    in_maps = _prep_inputs(x, W)
    res = run_bass_kernel_spmd(nc, in_maps, list(range(NCORES)))
    return _postprocess([res.results[k]["v_out"] for k in range(NCORES)])
